# revision 1
# baseline (speedup 1.0000x reference)
"""Balanced BCE loss on 8 Trainium2 NeuronCores.

loss = -sum_i [ beta_i * sum_j(t_ij * ln(p_ij))
                + (1-beta_i) * sum_j((1-t_ij) * ln(1-p_ij)) ]
beta_i = 1 - mean_j(t_ij)

Per-core row statistics (8 batch rows per core):
  S=sum(t)  A=sum(t*lnp)  C=sum(t*ln1mp)  B=sum(ln1mp)
host combines: loss = -sum_rows[ beta*A + (1-beta)*(B-C) ], beta = 1-S/N

Engine assignment per row tile [128, 2048]:
  - ACT: lnp = Ln(p) bf16; ln1mp = Ln(1-p) bf16 with accum_out -> B per partition
  - DVE: cast t->bf16 (2x mode); m1 = t*lnp, m2 = t*ln1mp (bf16 TT, 2x mode)
  - PE: ones-matmul chunk reductions of m1/m2 and tb (bf16), plus one
        tiny matmul for the cross-partition reduce of B
"""

from contextlib import ExitStack

import numpy as np

import concourse.bass as bass
import concourse.mybir as mybir
import concourse.tile as tile
from concourse import bacc
from concourse.bass_utils import run_bass_kernel_spmd

B, N = 64, 262144
NCORES = 8
ROWS = B // NCORES  # rows per core
P = 128  # SBUF partitions

AF = mybir.ActivationFunctionType
ALU = mybir.AluOpType
f32 = mybir.dt.float32
bf16 = mybir.dt.bfloat16

# test.py can flip this to capture an NTFF profile of the run
TRACE = False
LAST = None  # BassKernelResults of the most recent kernel() call


def _emit(tc, out_ap, inp_ap, tgt_ap, rows, n):
    """Emit the per-core program. out_ap: [1, 4*rows] f32 = [S.., B.., A.., C..]."""
    nc = tc.nc
    F = n // P
    CH = 128  # matmul moving-dim chunk; per-row PSUM stripe is CH wide
    nch = F // CH
    assert nch * CH == F

    with ExitStack() as ctx:
        io_pool = ctx.enter_context(tc.tile_pool(name="io", bufs=4))
        bf_pool = ctx.enter_context(tc.tile_pool(name="bf", bufs=3))
        psum_pool = ctx.enter_context(tc.tile_pool(name="ps", bufs=1, space="PSUM"))
        singles = ctx.enter_context(tc.tile_pool(name="const", bufs=1))

        ones_bf = singles.tile([P, 1], bf16, tag="ones_bf")
        nc.vector.memset(ones_bf[:], 1.0)
        ones_f = singles.tile([P, 1], f32, tag="ones_f")
        nc.vector.memset(ones_f[:], 1.0)
        accB = singles.tile([P, rows], f32, tag="accB")
        stats = singles.tile([1, 4 * rows], f32, tag="stats")
        junk = singles.tile([1, 128], f32, tag="junk")

        inp3 = inp_ap.rearrange("r (p f) -> p r f", p=P)
        tgt3 = tgt_ap.rearrange("r (p f) -> p r f", p=P)

        # psA/psC/psS: 2 PSUM banks each; psB gets its own bank
        psA = psum_pool.tile([1, rows * CH], f32, tag="psA", name="psA")
        psC = psum_pool.tile([1, rows * CH], f32, tag="psC", name="psC")
        psS = psum_pool.tile([1, rows * CH], f32, tag="psS", name="psS")
        psB = psum_pool.tile([1, rows], f32, tag="psB", name="psB")

        # per-row 1MB loads on the otherwise-idle SP engine, all triggers
        # emitted upfront (first io_bufs rows stream immediately; later
        # triggers wait inline on slot recycling, which only stalls SP).
        # The last t row is split in half so its consumer chain starts
        # ~1us earlier.
        ptiles, ttiles = [], []
        for r in range(rows):
            pp = io_pool.tile([P, F], f32, tag="p", name=f"pp_{r}")
            nc.sync.dma_start(pp[:], inp3[:, r, :])
            ptiles.append(pp)
            tt = io_pool.tile([P, F], f32, tag="t", name=f"tt_{r}")
            nc.sync.dma_start(tt[:], tgt3[:, r, :])
            ttiles.append(tt)

        for r in range(rows):
            p_t = ptiles[r][:]
            t_t = ttiles[r][:]

            logp = bf_pool.tile([P, F], bf16, tag="logp")
            nc.scalar.activation(logp[:], p_t, AF.Ln)
            l1mp = bf_pool.tile([P, F], bf16, tag="l1mp")
            nc.scalar.activation(
                l1mp[:], p_t, AF.Ln, scale=-1.0, bias=1.0,
                accum_out=accB[:, r : r + 1],
            )

            tb = bf_pool.tile([P, F], bf16, tag="tb")
            nc.vector.tensor_copy(tb[:], t_t)
            m1 = bf_pool.tile([P, F], bf16, tag="m1")
            nc.vector.tensor_mul(m1[:], tb[:], logp[:])
            m2 = bf_pool.tile([P, F], bf16, tag="m2")
            nc.vector.tensor_mul(m2[:], tb[:], l1mp[:])

            for ps, src in ((psS, tb), (psA, m1), (psC, m2)):
                for c in range(nch):
                    nc.tensor.matmul(
                        ps[0:1, r * CH : (r + 1) * CH],
                        ones_bf[:],
                        src[:, c * CH : (c + 1) * CH],
                        start=(c == 0),
                        stop=(c == nch - 1),
                    )

            # per-row second-level reduce on ACT (has slack; closer to
            # PSUM) via Copy+accum_out, overlapping later rows' stream
            for ps, col in ((psS, r), (psA, 2 * rows + r), (psC, 3 * rows + r)):
                nc.scalar.activation(
                    junk[0:1, :CH],
                    ps[0:1, r * CH : (r + 1) * CH],
                    AF.Copy,
                    accum_out=stats[0:1, col : col + 1],
                )

        # cross-partition reduce of B accumulators on PE
        nc.tensor.matmul(psB[0:1, :], ones_f[:], accB[:, :])
        nc.vector.tensor_copy(stats[0:1, rows : 2 * rows], psB[0:1, :])
        nc.sync.dma_start(out_ap, stats[:])


_PROG_CACHE = {}


def _build_program(rows=ROWS, n=N):
    key = (rows, n)
    if key not in _PROG_CACHE:
        nc = bacc.Bacc("TRN2", target_bir_lowering=False, debug=False)
        inp = nc.dram_tensor("input", [rows, n], f32, kind="ExternalInput").ap()
        tgt = nc.dram_tensor("target", [rows, n], f32, kind="ExternalInput").ap()
        out = nc.dram_tensor("partials", [1, 4 * rows], f32, kind="ExternalOutput").ap()
        with tile.TileContext(nc) as tc:
            _emit(tc, out, inp, tgt, rows, n)
        nc.finalize()
        _PROG_CACHE[key] = nc
    return _PROG_CACHE[key]


def kernel(input, target):
    global LAST
    input = np.ascontiguousarray(np.asarray(input))
    target = np.ascontiguousarray(np.asarray(target))
    assert input.shape == (B, N) and target.shape == (B, N)

    nc = _build_program()
    in_maps = [
        {
            "input": input[c * ROWS : (c + 1) * ROWS],
            "target": target[c * ROWS : (c + 1) * ROWS],
        }
        for c in range(NCORES)
    ]
    res = run_bass_kernel_spmd(nc, in_maps, core_ids=list(range(NCORES)), trace=TRACE)
    LAST = res

    total = np.float64(0.0)
    for c in range(NCORES):
        part = res.results[c]["partials"].astype(np.float64).reshape(4, ROWS)
        S, Bv, A, C = part[0], part[1], part[2], part[3]
        beta = 1.0 - S / N
        total += np.sum(beta * A + (1.0 - beta) * (Bv - C))
    return np.float32(-total)



# revision 6
# speedup vs baseline: 1.2386x; 1.2386x over previous
"""Balanced BCE loss on 8 Trainium2 NeuronCores.

loss = -sum_i [ beta_i * sum_j(t_ij * ln(p_ij))
                + (1-beta_i) * sum_j((1-t_ij) * ln(1-p_ij)) ]
beta_i = 1 - mean_j(t_ij)

Host casts inputs to bf16 (halves HBM traffic; ln error stays ~1e-6
relative on the summed loss) and reshapes each core's 8 rows to a flat
[128, 16384] layout where row r owns partitions 16r..16r+15.

Per-core row statistics (8 batch rows per core):
  S=sum(t)  A=sum(t*lnp)  C=sum(t*ln1mp)  B=sum(ln1mp)
host combines: loss = -sum_rows[ beta*A + (1-beta)*(B-C) ], beta = 1-S/N

Engine assignment per column chunk [128, F]:
  - ACT: lnp = Ln(p) bf16; ln1mp = Ln(1-p) bf16 with accum_out -> accB
         (ACT is the bottleneck: 2 passes = ~27.3us at 1 elem/lane/cycle)
  - DVE: m1 = t*lnp, m2 = t*ln1mp (bf16 TT, 2x mode)
  - PE: selector-matrix matmuls W[128,8]^T @ src accumulate S/A/C for
        all 8 rows at once across all chunks (single PSUM accumulation
        group per stat; one small final reduce each on DVE)
"""

from contextlib import ExitStack

import numpy as np
import ml_dtypes

import concourse.bass as bass
import concourse.mybir as mybir
import concourse.tile as tile
from concourse import bacc
from concourse.bass_utils import run_bass_kernel_spmd

B, N = 64, 262144
NCORES = 8
ROWS = B // NCORES  # rows per core
P = 128  # SBUF partitions
NF = ROWS * N // P  # 16384 free-dim cols per partition
PPR = P // ROWS  # 16 partitions per row

AF = mybir.ActivationFunctionType
ALU = mybir.AluOpType
AX = mybir.AxisListType
f32 = mybir.dt.float32
bf16 = mybir.dt.bfloat16
np_bf16 = ml_dtypes.bfloat16

CH = 512  # PSUM accumulator width / matmul moving window
# graduated chunk grid: small first chunk starts ACT early, small last
# chunk keeps the post-stream tail short
CHUNKS = [2048, 4608, 4608, 4096, 1024]
assert sum(CHUNKS) == NF and all(c % CH == 0 for c in CHUNKS)

# test.py can flip this to capture an NTFF profile of the run
TRACE = False
LAST = None  # BassKernelResults of the most recent kernel() call


def _emit(tc, out_ap, inp_ap, tgt_ap, wbf_ap, wf_ap):
    nc = tc.nc
    nch = len(CHUNKS)
    offs = [sum(CHUNKS[:i]) for i in range(nch)]
    nwin_total = NF // CH

    with ExitStack() as ctx:
        singles = ctx.enter_context(tc.tile_pool(name="s", bufs=1))
        psum_pool = ctx.enter_context(tc.tile_pool(name="ps", bufs=1, space="PSUM"))

        # selector matrices: W[p, r] = 1 iff p // 16 == r (host-provided;
        # loaded on the scalar HWDGE queue so the input stream on the SP
        # queue is not delayed)
        wbf = singles.tile([P, ROWS], bf16, tag="wbf")
        nc.scalar.dma_start(wbf[:], wbf_ap)
        wf = singles.tile([P, ROWS], f32, tag="wf")
        nc.scalar.dma_start(wf[:], wf_ap)

        accB = singles.tile([P, nch], f32, tag="accB")
        stats = singles.tile([ROWS, 4], f32, tag="stats")

        psS = psum_pool.tile([ROWS, CH], f32, tag="psS", name="psS")
        psA = psum_pool.tile([ROWS, CH], f32, tag="psA", name="psA")
        psC = psum_pool.tile([ROWS, CH], f32, tag="psC", name="psC")
        psB2 = psum_pool.tile([ROWS, nch], f32, tag="psB2", name="psB2")

        # stage all input loads upfront on the SP queue; order prioritizes
        # p chunks (ACT's critical path) one chunk ahead of t
        ptiles = [singles.tile([P, F], bf16, tag=f"p{c}", name=f"p{c}") for c, F in enumerate(CHUNKS)]
        ttiles = [singles.tile([P, F], bf16, tag=f"t{c}", name=f"t{c}") for c, F in enumerate(CHUNKS)]
        order = [("p", 0), ("p", 1), ("t", 0), ("p", 2), ("t", 1), ("p", 3),
                 ("t", 2), ("p", 4), ("t", 3), ("t", 4)]
        for kind, c in order:
            src = inp_ap if kind == "p" else tgt_ap
            dst = ptiles[c] if kind == "p" else ttiles[c]
            nc.sync.dma_start(dst[:], src[:, offs[c] : offs[c] + CHUNKS[c]])

        win = 0
        for c, F in enumerate(CHUNKS):
            p_t = ptiles[c][:]
            t_t = ttiles[c][:]

            lnp = singles.tile([P, F], bf16, tag=f"lnp{c}", name=f"lnp{c}")
            nc.scalar.activation(lnp[:], p_t, AF.Ln)
            l1mp = singles.tile([P, F], bf16, tag=f"l1mp{c}", name=f"l1mp{c}")
            nc.scalar.activation(
                l1mp[:], p_t, AF.Ln, scale=-1.0, bias=1.0,
                accum_out=accB[:, c : c + 1],
            )

            m1 = singles.tile([P, F], bf16, tag=f"m1{c}", name=f"m1{c}")
            nc.vector.tensor_mul(m1[:], t_t, lnp[:])
            m2 = singles.tile([P, F], bf16, tag=f"m2{c}", name=f"m2{c}")
            nc.vector.tensor_mul(m2[:], t_t, l1mp[:])

            for j in range(F // CH):
                sl = slice(j * CH, (j + 1) * CH)
                first = win == 0
                last = win == nwin_total - 1
                nc.tensor.matmul(psS[:, :], wbf[:], t_t[:, sl], start=first, stop=last)
                nc.tensor.matmul(psA[:, :], wbf[:], m1[:, sl], start=first, stop=last)
                nc.tensor.matmul(psC[:, :], wbf[:], m2[:, sl], start=first, stop=last)
                win += 1

        # final reduces on DVE (idle at tail time); B's cross-partition
        # fold on PE first
        nc.vector.tensor_reduce(stats[:, 0:1], psS[:, :], axis=AX.X, op=ALU.add)
        nc.vector.tensor_reduce(stats[:, 2:3], psA[:, :], axis=AX.X, op=ALU.add)
        nc.vector.tensor_reduce(stats[:, 3:4], psC[:, :], axis=AX.X, op=ALU.add)
        nc.tensor.matmul(psB2[:, :], wf[:], accB[:, :])
        nc.vector.tensor_reduce(stats[:, 1:2], psB2[:, :], axis=AX.X, op=ALU.add)
        nc.sync.dma_start(out_ap, stats[:])


_PROG_CACHE = {}


def _build_program():
    key = "v3"
    if key not in _PROG_CACHE:
        nc = bacc.Bacc("TRN2", target_bir_lowering=False, debug=False)
        inp = nc.dram_tensor("input", [P, NF], bf16, kind="ExternalInput").ap()
        tgt = nc.dram_tensor("target", [P, NF], bf16, kind="ExternalInput").ap()
        wbf_d = nc.dram_tensor("wsel_bf", [P, ROWS], bf16, kind="ExternalInput").ap()
        wf_d = nc.dram_tensor("wsel_f32", [P, ROWS], f32, kind="ExternalInput").ap()
        out = nc.dram_tensor("partials", [ROWS, 4], f32, kind="ExternalOutput").ap()
        with tile.TileContext(nc) as tc:
            _emit(tc, out, inp, tgt, wbf_d, wf_d)
        nc.finalize()
        _PROG_CACHE[key] = nc
    return _PROG_CACHE[key]


def kernel(input, target):
    global LAST
    input = np.asarray(input)
    target = np.asarray(target)
    assert input.shape == (B, N) and target.shape == (B, N)

    inp_bf = np.ascontiguousarray(input).astype(np_bf16)
    tgt_bf = np.ascontiguousarray(target).astype(np_bf16)

    nc = _build_program()
    wsel = np.zeros((P, ROWS), dtype=np.float32)
    for r in range(ROWS):
        wsel[r * PPR : (r + 1) * PPR, r] = 1.0
    wsel_bf = wsel.astype(np_bf16)
    in_maps = [
        {
            "input": inp_bf[c * ROWS : (c + 1) * ROWS].reshape(P, NF),
            "target": tgt_bf[c * ROWS : (c + 1) * ROWS].reshape(P, NF),
            "wsel_bf": wsel_bf,
            "wsel_f32": wsel,
        }
        for c in range(NCORES)
    ]
    res = run_bass_kernel_spmd(nc, in_maps, core_ids=list(range(NCORES)), trace=TRACE)
    LAST = res

    total = np.float64(0.0)
    for c in range(NCORES):
        part = res.results[c]["partials"].astype(np.float64)  # [ROWS, 4]
        S, Bv, A, C = part[:, 0], part[:, 1], part[:, 2], part[:, 3]
        beta = 1.0 - S / N
        total += np.sum(beta * A + (1.0 - beta) * (Bv - C))
    return np.float32(-total)
